# revision 3
# baseline (speedup 1.0000x reference)
"""GPT-2 style attention block (B=8, S=1024, NX=1024, H=16, D=64) on 8 TRN2
NeuronCores, data-parallel over batch (one batch element per core).

Per-core math (batch element b):
  qkv = x @ w_attn + b_attn ; split q,k,v ; per head: softmax(causal(q k^T / 8)) v
  out = merge_heads @ w_proj + b_proj

v2 layout/pipeline strategy (single core, no collectives):
  - xT built via the DMA XBAR transpose (bf16) -- zero PE time.
  - v computed FIRST (natural layout [sk, (h, d|1)] with an all-ones column
    per head so the PV matmul yields the softmax denominator for free).
  - The q/k projection is then INTERLEAVED with attention, head-pair by
    head-pair: while ACT exponentiates pair t's scores the PE computes the
    q/k tiles of pair t+1 and the PV matmuls of pair t-1, so the tensor
    engine never idles (keeps the HAM clock-gate at 8/8) and the exp
    latency is fully hidden.
  - Scores are computed transposed (ST[sk, sq]) with the two heads of a
    pair on disjoint PE row-groups (concurrent matmuls); both heads' score
    chunks land in one PSUM slot so a single wide ACT exp covers the pair.
  - Normalization: denominator row -> repartitioned reciprocal -> PE
    outer-product broadcast for both heads at once -> one DVE multiply.
  - ACT does ONLY exp + tiny PSUM evictions; all weight f32->bf16 casts run
    on GpSimd; PSUM evictions with bias-add run on DVE (tensor_scalar).

All matmuls run in bf16 (fp32 PSUM accumulation); rel err ~4e-3 vs the
fp32 reference.
"""

import numpy as np

B, S, NX, H = 8, 1024, 1024, 16
D = NX // H          # 64
P = 128              # partitions
ST = S // P          # 8 s-tiles
KT = NX // P         # 8 k-tiles
CH = 512             # matmul free-dim chunk (one PSUM bank of fp32)
NCH = S // CH        # 2 chunks
E = D + 1            # v columns per head incl. ones column
NPAIR = H // 2       # 8 head pairs


def _split_excess_waits(nc):
    """Post-scheduling pass: the TPB instruction encodings carry at most one
    embedded sync-wait (and matmuls with their fused weight-load carry none),
    but Tile may attach several.  Move excess waits onto InstNoOp instructions
    inserted immediately before, on the same engine."""
    import concourse.mybir as mybir

    SKIP = {
        "InstEventSemaphore",
        "InstUnconditionalBranch",
        "InstConditionalBranch",
        "InstRegisterMove",
        "InstRegisterAluOp",
    }
    n = 0
    for fn in nc.m.functions:
        for bb in fn.blocks:
            insts = bb.instructions
            inserts = []  # (index, [nops])
            for i, inst in enumerate(insts):
                tname = type(inst).__name__
                if tname in SKIP:
                    continue
                si = inst.sync_info
                if si is None or not si.on_wait:
                    continue
                waits = list(si.on_wait)
                cap = 1
                if len(waits) <= cap:
                    continue
                keep, move = waits[:cap], waits[cap:]
                nops = []
                for w in move:
                    n += 1
                    nops.append(
                        mybir.InstNoOp(
                            name=f"wsplit-{n}",
                            text_hint="wsplit",
                            bass_nofuse=True,
                            engine=inst.engine,
                            sync_info=mybir.SyncInfo(on_wait=[w], on_update=[]),
                        )
                    )
                inst.sync_info = mybir.SyncInfo(
                    on_wait=keep,
                    on_update=list(si.on_update) if si.on_update else [],
                )
                inserts.append((i, nops))
            for i, nops in reversed(inserts):
                for nop in reversed(nops):
                    insts.insert(i, nop)
                    try:
                        nc.register_instruction(nop, overwrite=True)
                    except Exception:
                        pass
    return n


def build_nc():
    import concourse.bass as bass
    import concourse.mybir as mybir
    from concourse.tile import TileContext
    from concourse.masks import make_upper_triangular

    f32 = mybir.dt.float32
    bf16 = mybir.dt.bfloat16
    Exp = mybir.ActivationFunctionType.Exp

    nc = bass.Bass(target_bir_lowering=False)
    x_ext = nc.declare_dram_parameter("x", [S, NX], f32, isOutput=False)
    wa_ext = nc.declare_dram_parameter("w_attn", [NX, 3 * NX], f32, isOutput=False)
    ba_ext = nc.declare_dram_parameter("b_attn", [3 * NX], f32, isOutput=False)
    wp_ext = nc.declare_dram_parameter("w_proj", [NX, NX], f32, isOutput=False)
    bp_ext = nc.declare_dram_parameter("b_proj", [NX], f32, isOutput=False)
    out_ext = nc.declare_dram_parameter("out", [S, NX], f32, isOutput=True)

    wa_r = wa_ext.rearrange("(kt p) n -> p kt n", p=P)

    with TileContext(nc) as tc:
        with (
            tc.tile_pool(name="const", bufs=1) as const,
            tc.tile_pool(name="small", bufs=2) as small,
            tc.tile_pool(name="persist", bufs=1) as persist,
            tc.tile_pool(name="qk", bufs=6) as qkp,
            tc.tile_pool(name="wpool", bufs=2) as wpool,
            tc.tile_pool(name="ps", bufs=1, space="PSUM") as ps,
        ):
            # ---------------- constants ----------------
            mask01 = const.tile([P, P], bf16)   # keep sq >= sk (incl diag)
            make_upper_triangular(nc, mask01, val=1.0, diag=True)
            ones_row = const.tile([1, P], bf16)
            nc.vector.memset(ones_row, 1.0)
            ba_v = const.tile([1, NX], bf16)    # b_attn[2048:3072] (v bias)
            nc.gpsimd.dma_start(out=ba_v, in_=ba_ext[2 * NX : 3 * NX].unsqueeze(0))
            ba_col = const.tile([P, 2 * KT], f32)  # b_attn[:2048] column-major
            nc.sync.dma_start(
                out=ba_col, in_=ba_ext[0 : 2 * NX].rearrange("(nt p) -> p nt", p=P)
            )
            bp_row = const.tile([1, NX], bf16)
            nc.gpsimd.dma_start(out=bp_row, in_=bp_ext[:].unsqueeze(0))

            # ---------------- persistent tiles ----------------
            xT = persist.tile([P, KT * S], bf16)       # 16KB/part
            v_sb = persist.tile([P, ST * H * E], bf16)  # 16.3KB
            aT = persist.tile([P, NPAIR * S], bf16)    # 16KB
            wp_sb = persist.tile([P, KT * NX], bf16)   # 16KB

            # ---------------- phase A: xT via XBAR transpose ----------------
            cm_early = tc.tile_pool(name="early", bufs=1)
            early = cm_early.__enter__()
            x_bf = early.tile([P, ST * NX], bf16)
            wv = early.tile([P, KT * NX], bf16)
            cm_stage = tc.tile_pool(name="stage", bufs=4)
            stage = cm_stage.__enter__()
            for st in range(ST):
                xs = stage.tile([P, NX], f32, name="xs")
                nc.sync.dma_start(
                    out=xs, in_=x_ext[st * P : (st + 1) * P, :]
                )
                nc.vector.tensor_copy(
                    out=x_bf[:, st * NX : (st + 1) * NX], in_=xs
                )
                nc.sync.dma_start_transpose(
                    out=bass.AP(
                        tensor=xT.tensor,
                        offset=xT.offset + st * P,
                        ap=[[KT * S, P], [S, KT], [1, P]],
                    ),
                    in_=x_bf[:, st * NX : (st + 1) * NX],
                )
                # interleave the w_v tile loads with the x tiles
                wvs = stage.tile([P, NX], f32, name="xs")
                nc.sync.dma_start(out=wvs, in_=wa_r[:, st, 2 * NX : 3 * NX])
                nc.gpsimd.tensor_copy(
                    out=wv[:, st * NX : (st + 1) * NX], in_=wvs
                )

            # ---------------- phase B2: v natural [sk, (h, d|1)] ----------------
            v_r = v_sb.rearrange("p (st h e) -> p st h e", h=H, e=E)
            nc.vector.memset(v_r[:, :, :, D : D + 1], 1.0)
            for st in range(ST):
                pm = ps.tile([P, NX], f32, name="pm", bufs=2)
                for kt in range(KT):
                    for c in range(NCH):
                        nc.tensor.matmul(
                            out=pm[:, c * CH : (c + 1) * CH],
                            lhsT=xT[:, kt * S + st * P : kt * S + (st + 1) * P],
                            rhs=wv[:, kt * NX + c * CH : kt * NX + (c + 1) * CH],
                            start=(kt == 0),
                            stop=False,
                        )
                for c in range(NCH):
                    nc.tensor.matmul(  # + b_attn[2048:] over all rows
                        out=pm[:, c * CH : (c + 1) * CH],
                        lhsT=ones_row,
                        rhs=ba_v[:, c * CH : (c + 1) * CH],
                        start=False,
                        stop=True,
                    )
                nc.vector.tensor_copy(
                    out=v_r[:, st, :, 0:D],
                    in_=pm.rearrange("p (h d) -> p h d", d=D),
                )
            cm_stage.__exit__(None, None, None)
            cm_early.__exit__(None, None, None)

            # ---------------- phase B||C: q/k projection + attention ----------
            qk_tiles = {}

            def emit_b_tile(nt):
                """q/k n-tile nt: qkT_nt[n, s] = (x @ w[:, nt])^T + b."""
                ws = wpool.tile([P, KT * P], f32, name="wstage")
                nc.sync.dma_start(
                    out=ws.rearrange("p (kt n) -> p kt n", n=P),
                    in_=wa_r[:, :, nt * P : (nt + 1) * P],
                )
                wsl = wpool.tile([P, KT * P], bf16, name="wsl", bufs=3)
                nc.gpsimd.tensor_copy(out=wsl, in_=ws)
                pm = ps.tile([P, S], f32, name="pm", bufs=2)
                for kt in range(KT):
                    for c in range(NCH):
                        nc.tensor.matmul(
                            out=pm[:, c * CH : (c + 1) * CH],
                            lhsT=wsl[:, kt * P : (kt + 1) * P],
                            rhs=xT[:, kt * S + c * CH : kt * S + (c + 1) * CH],
                            start=(kt == 0),
                            stop=(kt == KT - 1),
                        )
                qt = qkp.tile([P, S], bf16, name="qkt")
                nc.vector.tensor_scalar_add(
                    out=qt, in0=pm, scalar1=ba_col[:, nt : nt + 1]
                )
                qk_tiles[nt] = qt

            cm_et = tc.tile_pool(name="pool_et", bufs=2)
            pool_et = cm_et.__enter__()

            state = {}

            def pv_mm_list():
                mms = []
                for hh in (0, 1):
                    for c in range(NCH):
                        kt_hi = min(KT, ((c + 1) * CH) // P)
                        for kt in range(kt_hi):
                            off = max(0, P * kt - c * CH)
                            mms.append(
                                (hh, c, kt, off, kt == 0, kt == kt_hi - 1)
                            )
                # alternate the two heads' PV matmuls to keep both ET halves hot
                a = [m for m in mms if m[0] == 0]
                b = [m for m in mms if m[0] == 1]
                inter = []
                for x_, y_ in zip(a, b):
                    inter.extend((x_, y_))
                return inter

            def emit_pv(t, ET, pus, chunk):
                for hh, c, kt, off, first, last in chunk:
                    h = 2 * t + hh
                    nc.tensor.matmul(
                        out=pus[hh][:, c * CH + off : (c + 1) * CH],
                        lhsT=v_sb[
                            :, (kt * H + h) * E : (kt * H + h) * E + E
                        ],
                        rhs=ET[
                            :,
                            hh * KT * S + kt * S + c * CH + off : hh * KT * S
                            + kt * S
                            + (c + 1) * CH,
                        ],
                        start=first,
                        stop=last,
                    )

            def emit_mask_half(ET, hh, lo, hi):
                # diagonal blocks kt in [lo, hi): one strided DVE multiply
                diag = bass.AP(
                    tensor=ET.tensor,
                    offset=ET.offset + hh * KT * S + lo * (S + P),
                    ap=[[2 * KT * S, P], [S + P, hi - lo], [1, P]],
                )
                nc.vector.tensor_mul(
                    out=diag,
                    in0=diag,
                    in1=mask01.unsqueeze(1).broadcast_to((P, hi - lo, P)),
                )

            def emit_finish(t, ET, pus):
                """unnormalized evict + denominator reciprocal + broadcast +
                one normalizing multiply into aT[:, t*S:(t+1)*S]."""
                # evict unnormalized numerators (frees the pu slots)
                aTp = small.tile([P, S], bf16, name="aTp")
                for hh in (0, 1):
                    nc.vector.tensor_copy(
                        out=aTp[hh * 64 : (hh + 1) * 64, :],
                        in_=pus[hh][0:D, :],
                    )
                # denominator rows -> SBUF (ScalarE is close to PSUM)
                zrows = []
                for hh in (0, 1):
                    zr = small.tile([1, S], bf16, name="zrow")
                    nc.scalar.copy(out=zr, in_=pus[hh][D : D + 1, :])
                    zrows.append(zr)
                # repartition both heads' denominators, one wide reciprocal
                zwide = small.tile([P, 2 * S // P], bf16, name="zwide")
                nc.sync.dma_start(out=zwide[:, 0 : S // P], in_=zrows[0])
                nc.sync.dma_start(out=zwide[:, S // P :], in_=zrows[1])
                rwide = small.tile([P, 2 * S // P], bf16, name="rwide")
                with nc.allow_low_precision(
                    reason="softmax denominators; bf16 ok at 2e-2 gate"
                ):
                    nc.vector.reciprocal(out=rwide, in_=zwide)
                rrow = small.tile([1, 2 * S], bf16, name="rrow")
                nc.sync.dma_start(out=rrow[:, 0:S], in_=rwide[:, 0 : S // P])
                nc.sync.dma_start(out=rrow[:, S:], in_=rwide[:, S // P :])
                # broadcast 1/Z to 64 rows per head via PE outer product
                pr = ps.tile([P, S], f32, name="pm", bufs=2)
                for hh in (0, 1):
                    for c in range(NCH):
                        nc.tensor.matmul(
                            out=pr[
                                hh * 64 : (hh + 1) * 64, c * CH : (c + 1) * CH
                            ],
                            lhsT=ones_row[:, 0:64],
                            rhs=rrow[:, hh * S + c * CH : hh * S + (c + 1) * CH],
                            start=True,
                            stop=True,
                        )
                recipB = small.tile([P, S], bf16, name="recipB")
                nc.scalar.copy(out=recipB, in_=pr)
                nc.vector.tensor_mul(
                    out=aT[:, t * S : (t + 1) * S], in0=aTp, in1=recipB
                )

            def emit_pair(t):
                ET = pool_et.tile([P, 2 * KT * S], bf16, name="ET")
                ET_r = ET.rearrange("p (hh k) -> p hh k", hh=2)
                prev = state.pop(t - 1, None)
                if prev is not None:
                    prev_ET, prev_mms = prev
                    prev_pus = [
                        ps.tile([E, S], f32, name="pu", bufs=2) for _ in (0, 1)
                    ]
                qt = qk_tiles[t]
                kk = qk_tiles[NPAIR + t]
                for kt in range(KT):
                    for c in range(kt * P // CH, NCH):
                        off = max(0, kt * P - c * CH)
                        pm2 = ps.tile([P, S], f32, name="pm", bufs=2)
                        for hh in (0, 1):
                            nc.tensor.matmul(
                                out=pm2[:, hh * CH + off : (hh + 1) * CH],
                                lhsT=kk[
                                    hh * 64 : (hh + 1) * 64,
                                    kt * P : (kt + 1) * P,
                                ],
                                rhs=qt[
                                    hh * 64 : (hh + 1) * 64,
                                    c * CH + off : (c + 1) * CH,
                                ],
                                start=True,
                                stop=True,
                            )
                        nc.scalar.activation(
                            out=ET_r[
                                :, :, kt * S + c * CH + off : kt * S + (c + 1) * CH
                            ],
                            in_=pm2.rearrange("p (hh n) -> p hh n", hh=2)[
                                :, :, off:CH
                            ],
                            func=Exp,
                            scale=0.125,
                        )
                        # interleave PV matmuls of the previous pair
                        if prev is not None:
                            take = 2
                            chunk, prev_mms = prev_mms[:take], prev_mms[take:]
                            emit_pv(t - 1, prev_ET, prev_pus, chunk)
                    if kt == 3 or kt == 7:
                        for hh in (0, 1):
                            emit_mask_half(ET, hh, kt - 3, kt + 1)
                if prev is not None:
                    emit_pv(t - 1, prev_ET, prev_pus, prev_mms)
                    emit_finish(t - 1, prev_ET, prev_pus)
                state[t] = (ET, pv_mm_list())

            emit_b_tile(0)
            emit_b_tile(NPAIR)
            for t in range(NPAIR):
                if t < NPAIR - 1:
                    emit_b_tile(t + 1)
                    emit_b_tile(NPAIR + t + 1)
                emit_pair(t)
                if t == NPAIR - 2:
                    # prefetch w_proj during the last pairs
                    for kt in range(KT):
                        wps = wpool.tile([P, NX], f32, name="wpstage")
                        nc.sync.dma_start(out=wps, in_=wp_ext.rearrange(
                            "(kt p) n -> p kt n", p=P)[:, kt, :])
                        nc.gpsimd.tensor_copy(
                            out=wp_sb[:, kt * NX : (kt + 1) * NX], in_=wps
                        )
            # drain the last pair
            last_ET, last_mms = state.pop(NPAIR - 1)
            last_pus = [ps.tile([E, S], f32, name="pu", bufs=2) for _ in (0, 1)]
            emit_pv(NPAIR - 1, last_ET, last_pus, last_mms)
            emit_finish(NPAIR - 1, last_ET, last_pus)
            cm_et.__exit__(None, None, None)

            # ---------------- phase D: out = a @ w_proj + b_proj ----------------
            for st in range(ST):
                pm = ps.tile([P, NX], f32, name="pm", bufs=2)
                for kt in range(KT):
                    for c in range(NCH):
                        nc.tensor.matmul(
                            out=pm[:, c * CH : (c + 1) * CH],
                            lhsT=aT[:, kt * S + st * P : kt * S + (st + 1) * P],
                            rhs=wp_sb[:, kt * NX + c * CH : kt * NX + (c + 1) * CH],
                            start=(kt == 0),
                            stop=False,
                        )
                for c in range(NCH):
                    nc.tensor.matmul(
                        out=pm[:, c * CH : (c + 1) * CH],
                        lhsT=ones_row,
                        rhs=bp_row[:, c * CH : (c + 1) * CH],
                        start=False,
                        stop=True,
                    )
                dst = small.tile([P, NX], f32, name="dstage")
                nc.vector.tensor_copy(out=dst, in_=pm)
                nc.sync.dma_start(
                    out=out_ext[st * P : (st + 1) * P, :],
                    in_=dst,
                )

    _split_excess_waits(nc)
    return nc


def _enable_ldw_opt():
    """walrus is invoked with --enable-ldw-opt=false on this path; turning it
    on lets codegen elide redundant LDWEIGHTS for back-to-back matmuls that
    share a stationary operand."""
    import concourse.bass_utils as bu

    if getattr(bu, "_ldw_opt_patched", False):
        return
    orig = bu.run_command

    def patched(cmd, **kw):
        cmd = [
            c.replace("--enable-ldw-opt=false", "--enable-ldw-opt=true")
            if isinstance(c, str)
            else c
            for c in cmd
        ]
        return orig(cmd, **kw)

    bu.run_command = patched
    bu._ldw_opt_patched = True


def run(inputs, trace=False, **kwargs):
    """Run the SPMD kernel on 8 cores; returns (output, BassKernelResults)."""
    from concourse.bass_utils import run_bass_kernel_spmd

    x = np.ascontiguousarray(np.asarray(inputs["x"], dtype=np.float32))
    w_attn = np.ascontiguousarray(np.asarray(inputs["w_attn"], dtype=np.float32))
    b_attn = np.ascontiguousarray(np.asarray(inputs["b_attn"], dtype=np.float32))
    w_proj = np.ascontiguousarray(np.asarray(inputs["w_proj"], dtype=np.float32))
    b_proj = np.ascontiguousarray(np.asarray(inputs["b_proj"], dtype=np.float32))

    nc = build_nc()
    in_maps = [
        {
            "x": x[b],
            "w_attn": w_attn,
            "b_attn": b_attn,
            "w_proj": w_proj,
            "b_proj": b_proj,
        }
        for b in range(B)
    ]
    res = run_bass_kernel_spmd(
        nc, in_maps, core_ids=list(range(B)), trace=trace, **kwargs
    )
    out = np.stack([res.results[i]["out"] for i in range(B)], axis=0)
    return out.astype(np.float32), res


def kernel(**inputs):
    out, _ = run(inputs)
    return out
